# revision 12
# baseline (speedup 1.0000x reference)
"""Trainium2 Bass kernel for nn_AttentionGate (Swin-3D window attention block).

64 independent 343-token windows -> 8 windows/core on 8 cores, no collectives.
Host does window partition + transpose to feature-major (channels on
partitions). Device: LN1 -> KV proj -> window attention (S^T orientation:
k on partitions, q on free; softmax without max-subtraction since logits are
O(1); rel-pos bias folded multiplicatively via a host-precomputed exp(bias)
table) -> proj -> residual -> LN2 -> MLP. bf16 matmuls, fp32 accum/residual.
Each core processes its 8 windows in two sequential 4-window halves (SBUF fit).

Exact host-side folds: ln gains into following weights; kv_b K-half cancels in
softmax (per-q factor); kv_b V-half folds into proj_b.
"""

import os
import sys
import numpy as np

sys.path.insert(0, "/opt/trn_rl_repo")

import ml_dtypes

BF = ml_dtypes.bfloat16

WS = 7
B, DD, HH, WWD, C, NH = 1, 28, 28, 28, 384, 12
HD = C // NH          # 32
N = WS ** 3           # 343 tokens/window
NWIN = (DD // WS) ** 3  # 64
NCORES = 8
WPC = NWIN // NCORES  # 8 windows/core
T = WPC * N           # 2744 tokens/core
TH = T // 2           # 1372 tokens/half
WH = WPC // 2         # 4 windows/half
JC = 3                # 128-channel chunks
SCALE = HD ** -0.5
EPS = 1e-5
H4 = 4 * C            # 1536
KS = (128, 128, 87)   # k-tile sizes within a window
KOF = (0, 128, 256)


def _rel_index(ws):
    coords = np.stack(np.meshgrid(np.arange(ws), np.arange(ws), np.arange(ws), indexing="ij"))
    cf = coords.reshape(3, -1)
    rel = (cf[:, :, None] - cf[:, None, :]).transpose(1, 2, 0) + (ws - 1)
    rel[:, :, 0] *= (2 * ws - 1) ** 2
    rel[:, :, 1] *= (2 * ws - 1)
    return rel.sum(-1)


REL_IDX = _rel_index(WS)


def _win_part(x):
    b, d, h, w, c = x.shape
    x = x.reshape(b, d // WS, WS, h // WS, WS, w // WS, WS, c)
    return x.transpose(0, 1, 3, 5, 2, 4, 6, 7).reshape(-1, WS ** 3, c)


def _win_rev(x, b, d, h, w):
    c = x.shape[-1]
    x = x.reshape(b, d // WS, h // WS, w // WS, WS, WS, WS, c)
    return x.transpose(0, 1, 4, 2, 5, 3, 6, 7).reshape(b, d, h, w, c)


_PROG = None


def _build_program():
    import concourse.bacc as bacc
    import concourse.tile as tile
    from concourse import mybir
    from contextlib import ExitStack

    f32 = mybir.dt.float32
    bf16 = mybir.dt.bfloat16
    AF = mybir.ActivationFunctionType
    OP = mybir.AluOpType

    nc = bacc.Bacc("TRN2", target_bir_lowering=False, debug=False, enable_asserts=False)

    # register const APs for the eps biases used by the Sqrt activations
    for val in (float(EPS), float(EPS * HD)):
        t = nc.alloc_sbuf_tensor(f"const-eps-{val}", [128, 1], f32)
        nc.gpsimd.memset(t.ap(), val)
        nc.const_aps.aps[(f32, val)] = t.ap()
    nc.all_engine_barrier()

    d_sum = nc.dram_tensor("sumT", [128, JC, T], f32, kind="ExternalInput").ap()
    d_skip = nc.dram_tensor("skipT", [128, JC, T], bf16, kind="ExternalInput").ap()
    d_xup = nc.dram_tensor("xupT", [128, JC, T], bf16, kind="ExternalInput").ap()
    d_wk = nc.dram_tensor("wk", [128, JC, C], bf16, kind="ExternalInput").ap()
    d_wv = nc.dram_tensor("wv", [128, JC, C], bf16, kind="ExternalInput").ap()
    d_wp = nc.dram_tensor("wp", [128, JC, C], bf16, kind="ExternalInput").ap()
    d_w1 = nc.dram_tensor("w1", [128, JC, H4], bf16, kind="ExternalInput").ap()
    d_w2 = nc.dram_tensor("w2", [128, NH, C], bf16, kind="ExternalInput").ap()
    d_bp = nc.dram_tensor("bp", [128, JC], f32, kind="ExternalInput").ap()
    d_b1 = nc.dram_tensor("b1", [128, NH], f32, kind="ExternalInput").ap()
    d_b2 = nc.dram_tensor("b2", [128, JC], f32, kind="ExternalInput").ap()
    d_E = nc.dram_tensor("E", [128, NH, 3, N], bf16, kind="ExternalInput").ap()
    d_out = nc.dram_tensor("out", [128, JC, T], f32, kind="ExternalOutput").ap()

    with tile.TileContext(nc) as tc, ExitStack() as ctx:
        sing = ctx.enter_context(tc.tile_pool(name="sing", bufs=1))
        half = ctx.enter_context(tc.tile_pool(name="half", bufs=1))
        zbf = ctx.enter_context(tc.tile_pool(name="zbf", bufs=2))
        eff = ctx.enter_context(tc.tile_pool(name="eff", bufs=2))
        sm = ctx.enter_context(tc.tile_pool(name="sm", bufs=1))
        sm2 = ctx.enter_context(tc.tile_pool(name="sm2", bufs=2))
        ab = ctx.enter_context(tc.tile_pool(name="ab", bufs=2))
        pp = ctx.enter_context(tc.tile_pool(name="pp", bufs=2))
        ppP = ctx.enter_context(tc.tile_pool(name="ppP", bufs=3))
        hp = ctx.enter_context(tc.tile_pool(name="hp", bufs=1))
        yp = ctx.enter_context(tc.tile_pool(name="yp", bufs=2))
        ps_mm = ctx.enter_context(tc.tile_pool(name="ps_mm", bufs=2, space="PSUM"))
        ps_S = ctx.enter_context(tc.tile_pool(name="ps_S", bufs=2, space="PSUM"))
        ps_od = ctx.enter_context(tc.tile_pool(name="ps_od", bufs=1, space="PSUM"))

        wk_sb = sing.tile([128, JC, C], bf16)
        wv_sb = sing.tile([128, JC, C], bf16)
        wp_sb = sing.tile([128, JC, C], bf16)
        w1_sb = sing.tile([128, JC, H4], bf16)
        w2_sb = sing.tile([128, NH, C], bf16)
        E_sb = sing.tile([128, NH, 3, N], bf16)
        bp_sb = sing.tile([128, JC], f32)
        b1_sb = sing.tile([128, NH], f32)
        b2_sb = sing.tile([128, JC], f32)
        ones_sb = sing.tile([128, 32], bf16)
        for dst, src in ((wk_sb, d_wk), (wv_sb, d_wv), (wp_sb, d_wp), (w1_sb, d_w1),
                         (w2_sb, d_w2), (E_sb, d_E), (bp_sb, d_bp), (b1_sb, d_b1),
                         (b2_sb, d_b2)):
            nc.sync.dma_start(out=dst[:], in_=src[:])
        nc.vector.memset(ones_sb[:], 1.0)

        def layer_norm(z_bf, a_out, b_out, inv_s2):
            """z_bf (128, JC, TH) bf16 -> a_out = s/sqrt(var+eps), b_out = -mean*a
            as (1, TH) bf16 rows; inv_s2 = 1/s^2 folds the extra scale s."""
            sums = sm.tile([1, 2, TH], f32, tag="sums")
            for st in range(4):
                sl = slice(st * 343, (st + 1) * 343)
                sq = pp.tile([128, JC, 343], bf16, tag="sq")
                for ci in range(JC):
                    nc.vector.tensor_mul(sq[:, ci, :], z_bf[:, ci, sl], z_bf[:, ci, sl])
                ps_a = ps_mm.tile([128, 512], f32, tag="mm")
                ps_b = ps_mm.tile([128, 512], f32, tag="mm")
                for ci in range(JC):
                    nc.tensor.matmul(ps_a[0:1, 0:343], ones_sb[:, 0:1], z_bf[:, ci, sl],
                                     start=(ci == 0), stop=(ci == JC - 1))
                for ci in range(JC):
                    nc.tensor.matmul(ps_b[0:1, 0:343], ones_sb[:, 0:1], sq[:, ci, :],
                                     start=(ci == 0), stop=(ci == JC - 1))
                nc.vector.tensor_copy(sums[0:1, 0, sl], ps_a[0:1, 0:343])
                nc.vector.tensor_copy(sums[0:1, 1, sl], ps_b[0:1, 0:343])
            u = sm.tile([1, TH], f32, tag="u")
            nc.vector.scalar_tensor_tensor(u[:], sums[0:1, 0, :], 1.0 / C, sums[0:1, 0, :],
                                           op0=OP.mult, op1=OP.mult)
            nc.vector.tensor_sub(u[:], sums[0:1, 1, :], u[:])
            sqv = sm.tile([1, TH], f32, tag="sqv")
            nc.scalar.activation(sqv[:], u[:], AF.Sqrt,
                                 bias=float(EPS * inv_s2), scale=float(inv_s2) / C)
            r = sm.tile([1, TH], f32, tag="r")
            nc.vector.reciprocal_approx_fast(r[:], sqv[:])
            nc.vector.tensor_copy(a_out[:], r[:])
            nc.vector.scalar_tensor_tensor(b_out[:], sums[0:1, 0, :], -1.0 / C, r[:],
                                           op0=OP.mult, op1=OP.mult)

        def apply_ln(z_bf, a_row, b_row, out_t):
            a_b = ab.tile([128, TH], bf16, tag="ab")
            b_b = ab.tile([128, TH], bf16, tag="ab")
            nc.gpsimd.partition_broadcast(a_b[:], a_row[:])
            nc.gpsimd.partition_broadcast(b_b[:], b_row[:])
            for ci in range(JC):
                nc.vector.tensor_mul(out_t[:, ci, :], z_bf[:, ci, :], a_b[:])
                nc.vector.tensor_add(out_t[:, ci, :], out_t[:, ci, :], b_b[:])

        for h in range(2):
            toks = slice(h * TH, (h + 1) * TH)
            sumT = half.tile([128, JC, TH], f32, tag="sumT")
            skip_bf = zbf.tile([128, JC, TH], bf16, tag="zbf")
            xup_bf = zbf.tile([128, JC, TH], bf16, tag="zbf")
            nc.sync.dma_start(out=sumT[:], in_=d_sum[:, :, toks])
            nc.sync.dma_start(out=skip_bf[:], in_=d_skip[:, :, toks])
            nc.sync.dma_start(out=xup_bf[:], in_=d_xup[:, :, toks])

            a1 = sm2.tile([1, TH], bf16, tag="arow")
            c1 = sm2.tile([1, TH], bf16, tag="brow")
            layer_norm(skip_bf, a1, c1, 1.0)
            sk_eff = eff.tile([128, JC, TH], bf16, tag="eff")
            apply_ln(skip_bf, a1, c1, sk_eff)

            a2 = sm2.tile([1, TH], bf16, tag="arow")
            c2 = sm2.tile([1, TH], bf16, tag="brow")
            layer_norm(xup_bf, a2, c2, float(HD))
            q_eff = eff.tile([128, JC, TH], bf16, tag="eff")
            apply_ln(xup_bf, a2, c2, q_eff)

            # kT = (sk_eff @ wk)^T feature-major; K-side bias dropped (cancels in softmax)
            kT = half.tile([128, JC, TH], bf16, tag="kT")
            for co in range(JC):
                for st in range(4):
                    sl = slice(st * 343, (st + 1) * 343)
                    ps = ps_mm.tile([128, 512], f32, tag="mm")
                    for ci in range(JC):
                        nc.tensor.matmul(ps[:, 0:343], wk_sb[:, ci, co * 128:(co + 1) * 128],
                                         sk_eff[:, ci, sl], start=(ci == 0), stop=(ci == JC - 1))
                    nc.vector.tensor_copy(kT[:, co, sl], ps[:, 0:343])

            # v token-major (window-aligned tiles); V-side bias folded into proj_b
            v_sb = half.tile([128, WH * 3, C], bf16, tag="v")
            for w in range(WH):
                for j in range(3):
                    msz = KS[j]
                    t0 = w * N + KOF[j]
                    ps = ps_mm.tile([128, 512], f32, tag="mm")
                    for ci in range(JC):
                        nc.tensor.matmul(ps[0:msz, 0:C], sk_eff[:, ci, t0:t0 + msz],
                                         wv_sb[:, ci, :], start=(ci == 0), stop=(ci == JC - 1))
                    nc.vector.tensor_copy(v_sb[0:msz, w * 3 + j, :], ps[0:msz, 0:C])

            # attention
            oT = half.tile([128, JC, TH], bf16, tag="oT")
            for jc in range(JC):
                for w in range(WH):
                    qs = slice(w * N, (w + 1) * N)
                    od = ps_od.tile([128, 2, 512], f32, tag="od")
                    for j in range(3):
                        ksz = KS[j]
                        kt0 = w * N + KOF[j]
                        for bb in range(2):
                            S = ps_S.tile([128, 2, 512], f32, tag="S")
                            for a in range(2):
                                hl = 2 * bb + a
                                nc.tensor.matmul(S[0:ksz, a, 0:N],
                                                 kT[32 * hl:32 * hl + 32, jc, kt0:kt0 + ksz],
                                                 q_eff[32 * hl:32 * hl + 32, jc, qs],
                                                 start=True, stop=True,
                                                 tile_position=(32 * hl, 0))
                            P = ppP.tile([128, 2, N], bf16, tag="P")
                            nc.scalar.activation(P[0:ksz, :, :], S[0:ksz, :, 0:N], AF.Exp)
                            Pe = ppP.tile([128, 2, N], bf16, tag="P")
                            nc.vector.tensor_mul(
                                Pe[0:ksz, :, :], P[0:ksz, :, :],
                                E_sb[0:ksz, 4 * jc + 2 * bb:4 * jc + 2 * bb + 2, j, :])
                            for a in range(2):
                                hl = 2 * bb + a
                                hh = 4 * jc + hl
                                nc.tensor.matmul(od[32 * hl:32 * hl + 32, 0, 0:N],
                                                 v_sb[0:ksz, w * 3 + j, 32 * hh:32 * hh + 32],
                                                 Pe[0:ksz, a, :], start=(j == 0), stop=(j == 2),
                                                 tile_position=(0, 32 * hl))
                            for a in range(2):
                                hl = 2 * bb + a
                                nc.tensor.matmul(od[32 * hl:32 * hl + 32, 1, 0:N],
                                                 ones_sb[0:ksz, 0:32],
                                                 Pe[0:ksz, a, :], start=(j == 0), stop=(j == 2),
                                                 tile_position=(0, 32 * hl))
                    rden = pp.tile([128, N], f32, tag="rden")
                    nc.vector.reciprocal_approx_fast(rden[:, :], od[:, 1, 0:N])
                    nc.vector.tensor_mul(oT[:, jc, qs], od[:, 0, 0:N], rden[:, :])

            # proj + residual into sumT (becomes x)
            for co in range(JC):
                for st in range(4):
                    sl = slice(st * 343, (st + 1) * 343)
                    ps = ps_mm.tile([128, 512], f32, tag="mm")
                    for ci in range(JC):
                        nc.tensor.matmul(ps[:, 0:343], wp_sb[:, ci, co * 128:(co + 1) * 128],
                                         oT[:, ci, sl], start=(ci == 0), stop=(ci == JC - 1))
                    nc.vector.scalar_tensor_tensor(sumT[:, co, sl], ps[:, 0:343],
                                                   bp_sb[:, co:co + 1], sumT[:, co, sl],
                                                   op0=OP.add, op1=OP.add)

            # LN2 + MLP
            x2_bf = zbf.tile([128, JC, TH], bf16, tag="zbf")
            for ci in range(JC):
                nc.vector.tensor_copy(x2_bf[:, ci, :], sumT[:, ci, :])
            a3 = sm2.tile([1, TH], bf16, tag="arow")
            c3 = sm2.tile([1, TH], bf16, tag="brow")
            layer_norm(x2_bf, a3, c3, 1.0)
            x2_eff = eff.tile([128, JC, TH], bf16, tag="eff")
            apply_ln(x2_bf, a3, c3, x2_eff)

            for st in range(4):
                sl = slice(st * 343, (st + 1) * 343)
                hT = hp.tile([128, NH, N], bf16, tag="hT")
                for ho in range(NH):
                    ps = ps_mm.tile([128, 512], f32, tag="mm")
                    for ci in range(JC):
                        nc.tensor.matmul(ps[:, 0:N], w1_sb[:, ci, ho * 128:(ho + 1) * 128],
                                         x2_eff[:, ci, sl], start=(ci == 0), stop=(ci == JC - 1))
                    nc.scalar.activation(hT[:, ho, :], ps[:, 0:N], AF.Gelu,
                                         bias=b1_sb[:, ho:ho + 1])
                y = yp.tile([128, JC, N], f32, tag="y")
                for co in range(JC):
                    ps = ps_mm.tile([128, 512], f32, tag="mm")
                    for hc in range(NH):
                        nc.tensor.matmul(ps[:, 0:N], w2_sb[:, hc, co * 128:(co + 1) * 128],
                                         hT[:, hc, :], start=(hc == 0), stop=(hc == NH - 1))
                    nc.vector.scalar_tensor_tensor(y[:, co, :], ps[:, 0:N],
                                                   b2_sb[:, co:co + 1], sumT[:, co, sl],
                                                   op0=OP.add, op1=OP.add)
                nc.sync.dma_start(
                    out=d_out[:, :, h * TH + st * 343:h * TH + (st + 1) * 343], in_=y[:])

    nc.compile()
    return nc


def _get_program():
    global _PROG
    if _PROG is None:
        _PROG = _build_program()
    return _PROG


def _fm(x):
    """(T, C) fp32 -> (128, JC, T) feature-major."""
    return np.ascontiguousarray(x.T.reshape(JC, 128, x.shape[0]).transpose(1, 0, 2))


def _cvec(v, cols):
    return np.ascontiguousarray(np.asarray(v, np.float32).reshape(cols, 128).T)


def _prep_shards(skip, x_up, ln1_g, ln1_b, kv_w, kv_b, rpb, proj_w, proj_b,
                 ln2_g, ln2_b, mlp_w1, mlp_b1, mlp_w2, mlp_b2):
    skip = np.asarray(skip, np.float32).reshape(B, DD, HH, WWD, C)
    x_up = np.asarray(x_up, np.float32).reshape(B, DD, HH, WWD, C)
    ln1_g = np.asarray(ln1_g, np.float32)
    ln1_b = np.asarray(ln1_b, np.float32)
    assert not np.any(ln1_b), "q-side ln1_b fold not implemented (0 in setup_inputs)"

    sw = _win_part(skip)
    xw = _win_part(x_up)
    sumw = sw + xw

    kv_w = np.asarray(kv_w, np.float32)
    kv_b_eff = np.asarray(kv_b, np.float32) + ln1_b @ kv_w
    kv_wg = ln1_g[:, None] * kv_w
    wk_f = kv_wg[:, :C] * (ln1_g * SCALE)[None, :]
    wv_f = kv_wg[:, C:]
    bv_f = kv_b_eff[C:]
    proj_w = np.asarray(proj_w, np.float32)
    bp_f = np.asarray(proj_b, np.float32) + bv_f @ proj_w
    ln2_g = np.asarray(ln2_g, np.float32)
    ln2_b = np.asarray(ln2_b, np.float32)
    mlp_w1 = np.asarray(mlp_w1, np.float32)
    w1_f = ln2_g[:, None] * mlp_w1
    b1_f = np.asarray(mlp_b1, np.float32) + ln2_b @ mlp_w1

    rpb = np.asarray(rpb, np.float32)
    bias = rpb[REL_IDX.reshape(-1)].reshape(N, N, NH)   # (q, k, h)
    ET = np.exp(bias.transpose(2, 1, 0))                # (h, k, q)
    E_dev = np.zeros((128, NH, 3, N), np.float32)
    for j in range(3):
        E_dev[0:KS[j], :, j, :] = ET[:, KOF[j]:KOF[j] + KS[j], :].transpose(1, 0, 2)

    def wmat(w, chunks, cols):
        return np.ascontiguousarray(
            np.asarray(w, np.float32).reshape(chunks, 128, cols).transpose(1, 0, 2)).astype(BF)

    common = {
        "wk": wmat(wk_f, JC, C),
        "wv": wmat(wv_f, JC, C),
        "wp": wmat(proj_w, JC, C),
        "w1": wmat(w1_f, JC, H4),
        "w2": wmat(np.asarray(mlp_w2, np.float32), NH, C),
        "bp": _cvec(bp_f, JC),
        "b1": _cvec(b1_f, NH),
        "b2": _cvec(np.asarray(mlp_b2, np.float32), JC),
        "E": E_dev.astype(BF),
    }
    shards = []
    for core in range(NCORES):
        wsl = slice(core * WPC, (core + 1) * WPC)
        shards.append(dict(common,
                           sumT=_fm(sumw[wsl].reshape(T, C)).astype(np.float32),
                           skipT=_fm(sw[wsl].reshape(T, C)).astype(BF),
                           xupT=_fm(xw[wsl].reshape(T, C)).astype(BF)))
    return shards


def _gather(outs):
    full = np.empty((NWIN, N, C), np.float32)
    for core, o in enumerate(outs):
        xT = np.asarray(o, np.float32).transpose(1, 0, 2).reshape(C, T)
        full[core * WPC:(core + 1) * WPC] = xT.T.reshape(WPC, N, C)
    x = _win_rev(full, B, DD, HH, WWD)
    return x.reshape(B, DD * HH * WWD, C)


def _install_ntff_hook():
    """Provide the antenv.axon_hooks module the container image lacks, wired to
    the injected libaxon_pjrt.so, so trace=True yields neuron-profile NTFFs."""
    import types
    if "antenv.axon_hooks" not in sys.modules:
        import antenv
        mod = types.ModuleType("antenv.axon_hooks")
        state = {"hook": None}
        mod.set_axon_ntff_profile_hook = lambda h: state.update(hook=h)
        mod.get_axon_ntff_profile_hook = lambda: state["hook"]
        sys.modules["antenv.axon_hooks"] = mod
        antenv.axon_hooks = mod
    import antenv.axon_hooks as ah
    if ah.get_axon_ntff_profile_hook() is None:
        if "/root/.axon_site" not in sys.path:
            sys.path.insert(0, "/root/.axon_site")
        from trn_agent_boot.trn_boot import _ntff_profile_via_ctypes
        ah.set_axon_ntff_profile_hook(_ntff_profile_via_ctypes("/opt/axon/libaxon_pjrt.so"))
    # keep profile artifacts local (no bucket in this container)
    import concourse.bass_utils as bu
    bu.upload_artifacts = lambda tmpdir: tmpdir


def kernel(skip, x_up, D=None, H=None, W=None, **kw):
    from concourse.bass_utils import run_bass_kernel_spmd
    shards = _prep_shards(skip, x_up, kw["ln1_g"], kw["ln1_b"], kw["kv_w"], kw["kv_b"],
                          kw["rpb"], kw["proj_w"], kw["proj_b"], kw["ln2_g"], kw["ln2_b"],
                          kw["mlp_w1"], kw["mlp_b1"], kw["mlp_w2"], kw["mlp_b2"])
    nc = _get_program()
    trace = bool(int(os.environ.get("BASS_KERNEL_TRACE", "0")))
    if trace:
        _install_ntff_hook()
    res = run_bass_kernel_spmd(nc, shards, core_ids=list(range(NCORES)), trace=trace)
    kernel.last_exec_time_ns = res.exec_time_ns
    kernel.last_results = res
    return _gather([r["out"] for r in res.results]).astype(np.float32)
